# revision 1
# baseline (speedup 1.0000x reference)
"""Causal attention (B=8, S=2048, D=H=768) on 8 trn2 NeuronCores.

Data-parallel over batch: core c computes batch c entirely on-chip, no
collectives.  All matmuls contract over the partition dim.

Key algebraic move: scores = (x Wq)(x Wk)^T = x (Wq Wk^T) x^T, with
M = Wq Wk^T precomputed on host (768x768, ~0.5 GFLOP — negligible).  That
folds the q AND k projections into ONE on-device projection t = x M, and
the scores' k-side operand becomes raw x^T, whose exact bf16 hi/lo splits
ship straight from the host.

Precision scheme (validated vs fp64 in numpy: ~4e-4 rel absmax):
  - t = x M and scores = t x^T use bf16 hi/lo splits with 3-term matmuls
    (hi*hi + hi*lo + lo*hi, fp32 PSUM accumulation) — ~17-bit effective
    mantissa at 3 PE cycles/row.
  - V projection, exp weights, transposes, and attn@V run in fp16
    (11-bit mantissa, 1 cycle/row, fast weight load).
  - softmax stats (rowmax, rowsum, reciprocal) in fp32.

Per-core pipeline:
  phase 1b (first — small DMAs get PE going fastest):
      V[s,h] = x^T-blocks (stationary) x Wv (moving), fp16
  phase 1a: tT = M (stationary) x xT (moving), split to bf16 hi/lo
  phase 2, per 128-row q-tile (descending, so the exposed tail chain is
    the smallest tile): scores strip [q, k<=q]; causal mask on diag block;
    rowmax; exp (ScalarE, bias=-rowmax, accum_out=rowsum) -> fp16;
    PE-transpose exp blocks -> expT [k,q]; out = sum_k expT x V; scale by
    1/rowsum.

Host side: shards x over batch, pre-transposes/splits, computes M,
replicates weights, gathers per-core outputs.
"""

from contextlib import ExitStack

import ml_dtypes
import numpy as np

import bass_rust
import concourse.mybir as mybir
import concourse.tile as tile
from concourse import bacc
from concourse.bass_utils import run_bass_kernel_spmd
from concourse.masks import make_causal_mask, make_identity

B, S, D, H = 8, 2048, 768, 768
N_CORES = 8
P = 128
DT = D // P   # 6 d-tiles
HT = H // P   # 6 h-tiles
ST = S // P   # 16 s-tiles

f32 = mybir.dt.float32
bf16 = mybir.dt.bfloat16
f16 = mybir.dt.float16


def _ceil_div(a, b):
    return (a + b - 1) // b


def build_nc():
    nc = bacc.Bacc(None)

    # all inputs ship pre-tiled from the host in exact SBUF layout
    # ([128 partitions, ...]) so every DMA line is fully contiguous
    xb_d = nc.declare_dram_parameter("xb", [P, DT, S], bf16, isOutput=False)
    xl_d = nc.declare_dram_parameter("xl", [P, DT, S], bf16, isOutput=False)
    x16_d = nc.declare_dram_parameter("x16", [S // 512, P, DT, 512], f16, isOutput=False)
    mb_d = nc.declare_dram_parameter("mb", [P, DT, D], bf16, isOutput=False)
    ml_d = nc.declare_dram_parameter("ml", [P, DT, D], bf16, isOutput=False)
    wv16_d = nc.declare_dram_parameter("wv16", [P, DT, H], f16, isOutput=False)
    out_d = nc.declare_dram_parameter("out", [S, H], f32, isOutput=True)

    with tile.TileContext(nc, pool_alloc_mode="queue") as tc, ExitStack() as ctx:
        persist = ctx.enter_context(tc.tile_pool(name="persist", bufs=1))
        t_b = persist.tile([P, DT, S], bf16)   # 24KB/part each
        t_l = persist.tile([P, DT, S], bf16)
        xb_s = persist.tile([P, DT, S], bf16)
        xl_s = persist.tile([P, DT, S], bf16)
        V = persist.tile([P, ST, H], f16)      # 24KB/part
        ident16 = persist.tile([P, P], f16)
        cmask = persist.tile([P, P], f32)
        make_identity(nc, ident16)
        make_causal_mask(nc, cmask, mask_val=-1e10)

        mpool = tc.alloc_tile_pool(name="p1a_m", bufs=1)
        m_b = mpool.tile([P, DT, D], bf16)
        m_l = mpool.tile([P, DT, D], bf16)

        # ---- phase 1b: V (fp16) — first; its small DMAs start PE fastest -
        with tc.tile_pool(name="p1b_w", bufs=1) as wvpool, \
             tc.tile_pool(name="p1b_x", bufs=4) as xfpool, \
             tc.tile_pool(name="p1b_ps", bufs=4, space="PSUM") as pp:
            wv_s = wvpool.tile([P, DT, H], f16)
            nc.sync.dma_start(out=wv_s, in_=wv16_d[:, :, :])
            bulk_anchor = None
            for sc in range(S // 512):
                xfc = xfpool.tile([P, DT, 512], f16, tag="xfc", name="xfc")
                nc.sync.dma_start(out=xfc, in_=x16_d[sc])
                if sc == 1:
                    # phase-1a/2 bulk loads (8.6MB) stream in behind V
                    # compute on the SWDGE rings; the explicit dep on the
                    # first V-chunk's last copy keeps them from saturating
                    # HBM while the latency-critical first chunks transfer
                    assert bulk_anchor is not None
                    for dst, src in ((xb_s, xb_d), (xl_s, xl_d),
                                     (m_b, mb_d), (m_l, ml_d)):
                        dma = nc.gpsimd.dma_start(out=dst, in_=src[:, :, :])
                        bass_rust.add_dep_helper(
                            dma.ins, bulk_anchor.ins, sync=True,
                            reason="bulk load waits for first V chunk")
                for sti in range(4):
                    off = sti * P
                    for hc in range(2):
                        ps = pp.tile([P, 384], f32, tag="psv", name="psv")
                        for dt_ in range(DT):
                            nc.tensor.matmul(
                                ps,
                                xfc[:, dt_, off:off + P],
                                wv_s[:, dt_, hc * 384:(hc + 1) * 384],
                                start=(dt_ == 0),
                                stop=(dt_ == DT - 1),
                            )
                        cp = nc.any.tensor_copy(
                            V[:, sc * 4 + sti, hc * 384:(hc + 1) * 384], ps)
                        if sc == 0 and sti == 0 and hc == 1:
                            bulk_anchor = cp

        # ---- phase 1a: tT = M^T-stationary x xT (bf16 hi/lo x3) ----------
        with tc.tile_pool(name="p1a_ps", bufs=4, space="PSUM") as pp:
            for sc in range(S // 512):
                for ht in range(HT):
                    ps = pp.tile([P, 512], f32, tag="ps", name="ps")
                    idx = 0
                    for dt_ in range(DT):
                        for wop, xop in ((m_b, xb_s), (m_b, xl_s), (m_l, xb_s)):
                            nc.tensor.matmul(
                                ps,
                                wop[:, dt_, ht * P:(ht + 1) * P],
                                xop[:, dt_, sc * 512:(sc + 1) * 512],
                                start=(idx == 0),
                                stop=(idx == 3 * DT - 1),
                            )
                            idx += 1
                    hi_slice = t_b[:, ht, sc * 512:(sc + 1) * 512]
                    nc.scalar.copy(hi_slice, ps)
                    nc.vector.tensor_sub(
                        t_l[:, ht, sc * 512:(sc + 1) * 512], ps, hi_slice)
        mpool.release()

        # ---- phase 2: attention ------------------------------------------
        with tc.tile_pool(name="p2_strip", bufs=3) as strip_pool, \
             tc.tile_pool(name="p2_exp", bufs=3) as exp_pool, \
             tc.tile_pool(name="p2_expT", bufs=3) as expT_pool, \
             tc.tile_pool(name="p2_stat", bufs=4) as stat_pool, \
             tc.tile_pool(name="p2_out", bufs=2) as out_pool, \
             tc.tile_pool(name="p2_ps_s", bufs=4, space="PSUM") as ps_s_pool, \
             tc.tile_pool(name="p2_ps_t", bufs=2, space="PSUM") as ps_t_pool, \
             tc.tile_pool(name="p2_ps_o", bufs=2, space="PSUM") as ps_o_pool:
            # qt 0/1 first: they only need the first t chunk, so their long
            # stats chains hide under the tail of phase 1a; then descending
            # so the exposed end-of-kernel chain belongs to a small tile
            for qt in [0, 1] + list(range(ST - 1, 1, -1)):
                L = qt + 1
                cols = L * P
                strip = strip_pool.tile([P, S], f32, tag="strip", name="strip")
                for nch in range(_ceil_div(cols, 512)):
                    w = min(512, cols - nch * 512)
                    ps = ps_s_pool.tile([P, 512], f32, tag="ps_s", name="ps_s")
                    idx = 0
                    for dt_ in range(DT):
                        for qop, kop in ((t_b, xb_s), (t_b, xl_s), (t_l, xb_s)):
                            nc.tensor.matmul(
                                ps[:, :w],
                                qop[:, dt_, qt * P:(qt + 1) * P],
                                kop[:, dt_, nch * 512:nch * 512 + w],
                                start=(idx == 0),
                                stop=(idx == 3 * DT - 1),
                            )
                            idx += 1
                    nc.vector.tensor_copy(
                        strip[:, nch * 512:nch * 512 + w], ps[:, :w])
                nc.vector.tensor_add(
                    strip[:, (L - 1) * P:cols],
                    strip[:, (L - 1) * P:cols],
                    cmask,
                )
                nrmax = stat_pool.tile([P, 1], f32, tag="nrmax", name="nrmax")
                nc.vector.tensor_reduce(
                    nrmax, strip[:, :cols],
                    axis=mybir.AxisListType.X, op=mybir.AluOpType.max,
                    negate=True,
                )
                rsum = stat_pool.tile([P, 1], f32, tag="rsum", name="rsum")
                exp16 = exp_pool.tile([P, S], f16, tag="exp16", name="exp16")
                nc.scalar.activation(
                    exp16[:, :cols], strip[:, :cols],
                    mybir.ActivationFunctionType.Exp,
                    bias=nrmax, scale=1.0, accum_out=rsum,
                )
                rinv = stat_pool.tile([P, 1], f32, tag="rinv", name="rinv")
                nc.vector.reciprocal(rinv, rsum)
                expT = expT_pool.tile([P, ST, P], f16, tag="expT", name="expT")
                for j in range(L):
                    pst = ps_t_pool.tile([P, P], f16, tag="ps_t", name="ps_t")
                    nc.tensor.transpose(
                        pst, exp16[:, j * P:(j + 1) * P], ident16)
                    nc.any.tensor_copy(expT[:, j, :], pst)
                out_sb = out_pool.tile([P, H], f32, tag="out_sb", name="out_sb")
                for hc in range(2):
                    pso = ps_o_pool.tile([P, 384], f32, tag="ps_o", name="ps_o")
                    for j in range(L):
                        nc.tensor.matmul(
                            pso,
                            expT[:, j, :],
                            V[:, j, hc * 384:(hc + 1) * 384],
                            start=(j == 0),
                            stop=(j == L - 1),
                        )
                    nc.vector.tensor_scalar_mul(
                        out_sb[:, hc * 384:(hc + 1) * 384], pso, rinv)
                nc.sync.dma_start(
                    out=out_d[qt * P:(qt + 1) * P, :], in_=out_sb)

    nc.finalize()
    return nc


_NC_CACHE = None


def _get_nc():
    global _NC_CACHE
    if _NC_CACHE is None:
        _NC_CACHE = build_nc()
    return _NC_CACHE


def _split_b16(a):
    hi = a.astype(ml_dtypes.bfloat16)
    lo = (a - hi.astype(np.float32)).astype(ml_dtypes.bfloat16)
    return hi, lo


def _tile_rows(a):
    """[D, N] -> [128, D//128, N] (partition-major SBUF layout)."""
    d, n = a.shape
    return np.ascontiguousarray(a.reshape(d // P, P, n).transpose(1, 0, 2))


def make_in_maps(x, Wq, Wk, Wv):
    M = (Wq.astype(np.float64) @ Wk.astype(np.float64).T).astype(np.float32)
    mb, ml = _split_b16(M)
    mb, ml = _tile_rows(mb), _tile_rows(ml)
    wv16 = _tile_rows(Wv.astype(np.float16))
    in_maps = []
    for c in range(N_CORES):
        xT = np.ascontiguousarray(x[c].T)
        xb, xl = _split_b16(xT)
        x16 = np.ascontiguousarray(
            xT.astype(np.float16).reshape(DT, P, S // 512, 512)
            .transpose(2, 1, 0, 3))
        in_maps.append({
            "xb": _tile_rows(xb), "xl": _tile_rows(xl), "x16": x16,
            "mb": mb, "ml": ml, "wv16": wv16,
        })
    return in_maps


def kernel(x, Wq, Wk, Wv):
    x = np.asarray(x, dtype=np.float32)
    Wq = np.asarray(Wq, dtype=np.float32)
    Wk = np.asarray(Wk, dtype=np.float32)
    Wv = np.asarray(Wv, dtype=np.float32)

    nc = _get_nc()
    in_maps = make_in_maps(x, Wq, Wk, Wv)
    res = run_bass_kernel_spmd(nc, in_maps, list(range(N_CORES)))
    out = np.stack([res.results[c]["out"] for c in range(N_CORES)], axis=0)
    return out.astype(np.float32)



# revision 3
# speedup vs baseline: 1.4145x; 1.4145x over previous
"""Causal attention (B=8, S=2048, D=H=768) on 8 trn2 NeuronCores.

Data-parallel over batch: core c computes batch c entirely on-chip, no
collectives.  All matmuls contract over the partition dim.

Key algebraic move: scores = (x Wq)(x Wk)^T = x (Wq Wk^T) x^T, with
M = Wq Wk^T precomputed on host (768x768).  That folds the q AND k
projections into ONE on-device projection t = x M, and the scores'
k-side operand becomes raw x^T.

Precision scheme (validated vs fp32 reference, ~6e-3 rel absmax in a
HW-calibrated noise sim; gate is 2e-2):
  - t = x M and scores = t x^T run as SINGLE-PASS float32r matmuls
    (fp32 operands in SBUF, ~2^-13.5 internal product truncation,
    1 PE cycle/row for moving>=256 -- same rate as fp16, ~2x the
    accuracy, no operand-rounding error since x/M/t stay fp32).
  - V projection also f32r (output stored fp16).
  - exp weights, transposes, and attn@V run in fp16.
  - softmax stats (rowmax, rowsum, reciprocal) in fp32.

This replaces the previous 3-pass bf16 hi/lo scheme, cutting PE work
per core from ~730k to ~374k cycles.

Per-core pipeline:
  phase 1b (first -- small DMAs get PE going fastest):
      V[s,h] = x^T-blocks (stationary) x Wv (moving), f32r -> f16
  phase 1a: tT = M (stationary) x xT (moving), f32r -> f32
  phase 2, per 128-row q-tile: scores strip [q, k<=q] f32r; causal mask
    on diag block; rowmax; exp (ScalarE, bias=-rowmax, accum_out=rowsum)
    -> fp16; PE-transpose exp blocks -> expT [k,q]; out = sum_k expT x V
    (fp16); scale by 1/rowsum.  Tile order starts with two small tiles
    that only need the first t chunk (hide under phase-1a tail) and ends
    with the smallest tile so the exposed end-of-kernel chain is short.

Host side: shards x over batch, pre-transposes/tiles, computes
M = Wq Wk^T in float64, replicates weights, gathers per-core outputs.
"""

from contextlib import ExitStack

import numpy as np

import bass_rust
import concourse.mybir as mybir
import concourse.tile as tile
from concourse import bacc
from concourse.bass_utils import run_bass_kernel_spmd
from concourse.masks import make_causal_mask, make_identity

B, S, D, H = 8, 2048, 768, 768
N_CORES = 8
P = 128
DT = D // P   # 6 d-tiles
HT = H // P   # 6 h-tiles
ST = S // P   # 16 s-tiles
SC = S // 512  # 4 column-chunks

f32 = mybir.dt.float32
f32r = mybir.dt.float32r
f16 = mybir.dt.float16


def _ceil_div(a, b):
    return (a + b - 1) // b


def build_nc():
    nc = bacc.Bacc(None)

    # inputs ship pre-tiled from the host in exact SBUF layout
    # ([128 partitions, ...]) so every DMA line is fully contiguous
    x_d = nc.declare_dram_parameter("xin", [SC, P, DT, 512], f32r, isOutput=False)
    m_d = nc.declare_dram_parameter("m", [P, DT, D], f32r, isOutput=False)
    wv_d = nc.declare_dram_parameter("wv", [2, P, DT, 384], f32r, isOutput=False)
    out_d = nc.declare_dram_parameter("out", [S, H], f32, isOutput=True)

    with tile.TileContext(nc, pool_alloc_mode="queue") as tc, ExitStack() as ctx:
        persist = ctx.enter_context(tc.tile_pool(name="persist", bufs=1))
        x_s = persist.tile([P, SC, DT, 512], f32r)  # 48KB/part
        t_s = persist.tile([P, HT, S], f32r)        # 48KB/part
        V = persist.tile([P, ST, H], f16)          # 24KB/part
        ident16 = persist.tile([P, P], f16)
        cmask = persist.tile([P, P], f32)
        make_identity(nc, ident16)
        make_causal_mask(nc, cmask, mask_val=-1e10)

        mpool = tc.alloc_tile_pool(name="p1a_m", bufs=1)
        m_s = mpool.tile([P, DT, D], f32r)

        # ---- phase 1b: V (f32r) -- first; its DMAs get PE going fastest --
        with tc.tile_pool(name="p1b_w", bufs=1) as wvpool, \
             tc.tile_pool(name="p1b_ps", bufs=4, space="PSUM") as pp:
            wv_s = wvpool.tile([P, 2, DT, 384], f32r)
            nc.sync.dma_start(out=wv_s[:, 0], in_=wv_d[0])
            nc.sync.dma_start(out=wv_s[:, 1], in_=wv_d[1])
            bulk_anchor = None
            for sc in range(SC):
                nc.sync.dma_start(out=x_s[:, sc], in_=x_d[sc])
                if sc == 1:
                    # M (2.25MB) streams in behind V compute on the SWDGE
                    # ring; the explicit dep on the first V-chunk's copy
                    # keeps it off HBM while the latency-critical first
                    # chunks transfer
                    assert bulk_anchor is not None
                    dma = nc.gpsimd.dma_start(out=m_s, in_=m_d[:, :, :])
                    bass_rust.add_dep_helper(
                        dma.ins, bulk_anchor.ins, sync=True,
                        reason="m load waits for first V chunk")
                for sti in range(4):
                    off = sti * P
                    for hc in range(2):
                        ps = pp.tile([P, 384], f32, tag="psv", name="psv")
                        for dt_ in range(DT):
                            nc.tensor.matmul(
                                ps,
                                x_s[:, sc, dt_, off:off + P],
                                wv_s[:, hc, dt_, :],
                                start=(dt_ == 0),
                                stop=(dt_ == DT - 1),
                            )
                        cp = nc.any.tensor_copy(
                            V[:, sc * 4 + sti, hc * 384:(hc + 1) * 384], ps)
                        if sc == 0 and sti == 0 and hc == 1:
                            bulk_anchor = cp

        # ---- phase 1a: tT = M (stationary) x xT (moving), f32r ----------
        with tc.tile_pool(name="p1a_ps", bufs=4, space="PSUM") as pp:
            for sc in range(SC):
                for ht in range(HT):
                    ps = pp.tile([P, 512], f32, tag="ps", name="ps")
                    for dt_ in range(DT):
                        nc.tensor.matmul(
                            ps,
                            m_s[:, dt_, ht * P:(ht + 1) * P],
                            x_s[:, sc, dt_, :],
                            start=(dt_ == 0),
                            stop=(dt_ == DT - 1),
                        )
                    nc.any.tensor_copy(t_s[:, ht, sc * 512:(sc + 1) * 512], ps)
        mpool.release()

        # ---- phase 2: attention ------------------------------------------
        with tc.tile_pool(name="p2_strip", bufs=3) as strip_pool, \
             tc.tile_pool(name="p2_exp", bufs=3) as exp_pool, \
             tc.tile_pool(name="p2_expT", bufs=3) as expT_pool, \
             tc.tile_pool(name="p2_stat", bufs=4) as stat_pool, \
             tc.tile_pool(name="p2_out", bufs=2) as out_pool, \
             tc.tile_pool(name="p2_ps_s", bufs=4, space="PSUM") as ps_s_pool, \
             tc.tile_pool(name="p2_ps_t", bufs=2, space="PSUM") as ps_t_pool, \
             tc.tile_pool(name="p2_ps_o", bufs=2, space="PSUM") as ps_o_pool:
            # qt 1,2 first: they only need the first t chunk, so their long
            # stats chains hide under the tail of phase 1a; then descending;
            # qt 0 last so the exposed end-of-kernel chain is the smallest
            for qt in [1, 2] + list(range(ST - 1, 2, -1)) + [0]:
                L = qt + 1
                cols = L * P
                strip = strip_pool.tile([P, S], f32, tag="strip", name="strip")
                for nch in range(_ceil_div(cols, 512)):
                    w = min(512, cols - nch * 512)
                    # f32r needs moving >= 256 for the 1-cycle/row rate;
                    # pad short tail chunks with throwaway columns
                    wp = max(w, 256)
                    ps = ps_s_pool.tile([P, 512], f32, tag="ps_s", name="ps_s")
                    for dt_ in range(DT):
                        nc.tensor.matmul(
                            ps[:, :wp],
                            t_s[:, dt_, qt * P:(qt + 1) * P],
                            x_s[:, nch, dt_, 0:wp],
                            start=(dt_ == 0),
                            stop=(dt_ == DT - 1),
                        )
                    nc.vector.tensor_copy(
                        strip[:, nch * 512:nch * 512 + w], ps[:, :w])
                nc.vector.tensor_add(
                    strip[:, (L - 1) * P:cols],
                    strip[:, (L - 1) * P:cols],
                    cmask,
                )
                nrmax = stat_pool.tile([P, 1], f32, tag="nrmax", name="nrmax")
                nc.vector.tensor_reduce(
                    nrmax, strip[:, :cols],
                    axis=mybir.AxisListType.X, op=mybir.AluOpType.max,
                    negate=True,
                )
                rsum = stat_pool.tile([P, 1], f32, tag="rsum", name="rsum")
                exp16 = exp_pool.tile([P, S], f16, tag="exp16", name="exp16")
                nc.scalar.activation(
                    exp16[:, :cols], strip[:, :cols],
                    mybir.ActivationFunctionType.Exp,
                    bias=nrmax, scale=1.0, accum_out=rsum,
                )
                rinv = stat_pool.tile([P, 1], f32, tag="rinv", name="rinv")
                nc.vector.reciprocal(rinv, rsum)
                expT = expT_pool.tile([P, ST, P], f16, tag="expT", name="expT")
                for j in range(L):
                    pst = ps_t_pool.tile([P, P], f16, tag="ps_t", name="ps_t")
                    nc.tensor.transpose(
                        pst, exp16[:, j * P:(j + 1) * P], ident16)
                    nc.any.tensor_copy(expT[:, j, :], pst)
                out_sb = out_pool.tile([P, H], f32, tag="out_sb", name="out_sb")
                for hc in range(2):
                    pso = ps_o_pool.tile([P, 384], f32, tag="ps_o", name="ps_o")
                    for j in range(L):
                        nc.tensor.matmul(
                            pso,
                            expT[:, j, :],
                            V[:, j, hc * 384:(hc + 1) * 384],
                            start=(j == 0),
                            stop=(j == L - 1),
                        )
                    nc.vector.tensor_scalar_mul(
                        out_sb[:, hc * 384:(hc + 1) * 384], pso, rinv)
                nc.sync.dma_start(
                    out=out_d[qt * P:(qt + 1) * P, :], in_=out_sb)

    nc.finalize()
    return nc


_NC_CACHE = None


def _get_nc():
    global _NC_CACHE
    if _NC_CACHE is None:
        _NC_CACHE = build_nc()
    return _NC_CACHE


def _tile_rows(a):
    """[D, N] -> [128, D//128, N] (partition-major SBUF layout)."""
    d, n = a.shape
    return np.ascontiguousarray(a.reshape(d // P, P, n).transpose(1, 0, 2))


def make_in_maps(x, Wq, Wk, Wv):
    M = (Wq.astype(np.float64) @ Wk.astype(np.float64).T).astype(np.float32)
    m_t = _tile_rows(M)
    wv_t = _tile_rows(Wv.astype(np.float32))
    wv_t = np.ascontiguousarray(
        wv_t.reshape(P, DT, 2, 384).transpose(2, 0, 1, 3))
    in_maps = []
    for c in range(N_CORES):
        xT = np.ascontiguousarray(x[c].T)  # [D, S]
        xc = np.ascontiguousarray(
            xT.reshape(DT, P, SC, 512).transpose(2, 1, 0, 3))
        in_maps.append({"xin": xc, "m": m_t, "wv": wv_t})
    return in_maps


def kernel(x, Wq, Wk, Wv):
    x = np.asarray(x, dtype=np.float32)
    Wq = np.asarray(Wq, dtype=np.float32)
    Wk = np.asarray(Wk, dtype=np.float32)
    Wv = np.asarray(Wv, dtype=np.float32)

    nc = _get_nc()
    in_maps = make_in_maps(x, Wq, Wk, Wv)
    res = run_bass_kernel_spmd(nc, in_maps, list(range(N_CORES)))
    out = np.stack([res.results[c]["out"] for c in range(N_CORES)], axis=0)
    return out.astype(np.float32)


# revision 8
# speedup vs baseline: 1.7883x; 1.2643x over previous
"""Causal attention (B=8, S=2048, D=H=768) on 8 trn2 NeuronCores.

Data-parallel over batch: core c computes batch c entirely on-chip, no
collectives.  All matmuls contract over the partition dim.

Key algebraic move: scores = (x Wq)(x Wk)^T = x (Wq Wk^T) x^T, with
M = Wq Wk^T precomputed on host (768x768).  That folds the q AND k
projections into ONE on-device projection t = x M, and the scores'
k-side operand becomes raw x^T.

Precision scheme (HW-validated):
  - V = x Wv and t = x M run in single-pass fp16 (operand rounding only;
    PE accumulates exactly in fp32 PSUM).  t is stored as fp32 (float32r).
  - scores = t x^T runs as a SINGLE-PASS float32r matmul: fp32 operands
    in SBUF, ~2^-13.5 internal product truncation, 1.5 PE cycles/row --
    kills both the fp16 store-rounding of t and the k-side x rounding.
  - exp weights, transposes, and attn@V run in fp16; softmax stats fp32.

Per-core pipeline:
  phase 1b: V[s,h] = x16-blocks (stationary) x Wv16 (moving), fp16.
    The two head-critical DMAs (wv half 0, x16 chunk 0) are triggered
    from the Tensor queue, which is idle at t=0 (the Sync queue spends
    ~8us on semaphore init first).  Bulk loads (x32 chunks, m16) ride
    the SWDGE ring gated behind the first V copies.
  phase 1a: tT = M16 (stationary) x x16T (moving), fp16 -> f32r store.
    s-chunks run REVERSED (3,2,1,0) so the first phase-2 tiles (high qt)
    see their t chunk earliest.
  phase 2, per 128-row q-tile, qt descending 15..0: scores strip
    [q, k<=q] f32r; one DVE op per 512-chunk computes
    strip = causal_mask - psum (masked slots +1e10); a min-reduce gives
    -rowmax; exp on ScalarE (scale=-1, bias=-max, accum_out=rowsum)
    -> fp16; PE-transpose exp in batches of 4 blocks
    per PSUM tile with one copy each; out = sum_k expT x V (fp16);
    scale by 1/rowsum into an fp16 pair-accumulator; one DMA per TWO
    q-tiles ([128, 2*768] f16, partition-major DRAM layout) keeps the
    per-line descriptor overhead amortized and the end-of-kernel DMA
    drain short.

Host side: shards x over batch, pre-transposes/tiles, computes
M = Wq Wk^T in float64, replicates weights, gathers + de-tiles outputs.
"""

from contextlib import ExitStack

import numpy as np

import bass_rust
import concourse.mybir as mybir
import concourse.tile as tile
from concourse import bacc
from concourse.bass_utils import run_bass_kernel_spmd
from concourse.masks import make_causal_mask, make_identity

B, S, D, H = 8, 2048, 768, 768
N_CORES = 8
P = 128
DT = D // P    # 6 d-tiles
HT = H // P    # 6 h-tiles
ST = S // P    # 16 s-tiles
SC = S // 512  # 4 column-chunks

f32 = mybir.dt.float32
f32r = mybir.dt.float32r
f16 = mybir.dt.float16

# "mixed": scores in f32r (x32 shipped), t stored f32r.  rel err ~9e-3.
# "f16":   scores in fp16 (no x32), t stored f16.        rel err ~1.25e-2.
SCHEME = "mixed"


def _ceil_div(a, b):
    return (a + b - 1) // b


def build_nc(scheme=SCHEME):
    mixed = scheme == "mixed"
    nc = bacc.Bacc(None)

    # inputs ship pre-tiled from the host in exact SBUF layout
    # ([128 partitions, ...]) so every DMA line is fully contiguous
    x16_d = nc.declare_dram_parameter("x16", [SC, P, DT, 512], f16, isOutput=False)
    m16_d = nc.declare_dram_parameter("m16", [P, DT, D], f16, isOutput=False)
    wv_d = nc.declare_dram_parameter("wv", [2, P, DT, 384], f16, isOutput=False)
    if mixed:
        x32_d = nc.declare_dram_parameter(
            "x32", [SC, P, DT, 512], f32r, isOutput=False)
    # partition-major output: host de-tiles [P, ST, H] -> [S, H]
    out_d = nc.declare_dram_parameter("out", [P, ST, H], f16, isOutput=True)

    t_dt = f32r if mixed else f16

    with tile.TileContext(nc, pool_alloc_mode="queue") as tc, ExitStack() as ctx:
        persist = ctx.enter_context(tc.tile_pool(name="persist", bufs=1))
        t_s = persist.tile([P, HT, S], t_dt)     # 48KB/part (24 if f16)
        V = persist.tile([P, ST, H], f16)        # 24KB/part
        ident16 = persist.tile([P, P], f16)
        # zcm = [512 zeros | 128-col causal mask]; slice [640-w:640] puts
        # the mask on the last 128 of a w-wide diag chunk, zeros elsewhere
        zcm = persist.tile([P, 640], f32)
        if mixed:
            x32_s = persist.tile([P, SC, DT, 512], f32r)  # 48KB/part

        p1pool = tc.alloc_tile_pool(name="p1", bufs=1)
        x16_s = p1pool.tile([P, SC, DT, 512], f16)  # 24KB/part
        m16_s = p1pool.tile([P, DT, D], f16)
        wv_s = p1pool.tile([P, 2, DT, 384], f16)

        # head-critical loads on the Activation queue (lighter init than
        # the Sync queue): first V matmul group needs exactly these two
        nc.sync.dma_start(out=wv_s[:, 0], in_=wv_d[0])
        nc.sync.dma_start(out=x16_s[:, 0], in_=x16_d[0])
        # near-term loads on the Sync queue
        nc.sync.dma_start(out=wv_s[:, 1], in_=wv_d[1])
        for sc in range(1, SC):
            nc.sync.dma_start(out=x16_s[:, sc], in_=x16_d[sc])

        make_identity(nc, ident16)
        nc.gpsimd.memset(zcm[:, 0:512], 0.0)
        make_causal_mask(nc, zcm[:, 512:640], mask_val=1e10)

        # ---- phase 1b: V = x16 (stationary) x Wv16 (moving) --------------
        with tc.tile_pool(name="p1b_ps", bufs=4, space="PSUM") as pp:
            bulk_anchor = None
            for sc in range(SC):
                if sc == 1:
                    # bulk loads (x32 24KB/part + m16) stream on the SWDGE
                    # ring once the head-critical transfers are done
                    assert bulk_anchor is not None
                    bulk = [(m16_s, m16_d[:, :, :])]
                    if mixed:
                        bulk += [(x32_s[:, c], x32_d[c]) for c in range(SC)]
                    for dst, src in bulk:
                        dma = nc.gpsimd.dma_start(out=dst, in_=src)
                        bass_rust.add_dep_helper(
                            dma.ins, bulk_anchor.ins, sync=True,
                            reason="bulk load waits for first V chunk")
                for hc in range(2):
                    for sti in range(4):
                        off = sti * P
                        ps = pp.tile([P, 384], f32, tag="psv", name="psv")
                        for dt_ in range(DT):
                            nc.tensor.matmul(
                                ps,
                                x16_s[:, sc, dt_, off:off + P],
                                wv_s[:, hc, dt_, :],
                                start=(dt_ == 0),
                                stop=(dt_ == DT - 1),
                            )
                        cp = nc.vector.tensor_copy(
                            V[:, sc * 4 + sti, hc * 384:(hc + 1) * 384], ps)
                        if sc == 0 and hc == 0 and sti == 3:
                            bulk_anchor = cp

        # ---- phase 1a: tT = M16 (stationary) x x16T (moving) -------------
        # reversed s-chunk order: phase 2 runs qt descending, so high-qt
        # tiles (which need the last t chunk for their stationary) unblock
        # right after the first chunk-group here
        with tc.tile_pool(name="p1a_ps", bufs=4, space="PSUM") as pp:
            for sc in range(SC - 1, -1, -1):
                for ht in range(HT):
                    ps = pp.tile([P, 512], f32, tag="ps", name="ps")
                    for dt_ in range(DT):
                        nc.tensor.matmul(
                            ps,
                            m16_s[:, dt_, ht * P:(ht + 1) * P],
                            x16_s[:, sc, dt_, :],
                            start=(dt_ == 0),
                            stop=(dt_ == DT - 1),
                        )
                    nc.scalar.copy(t_s[:, ht, sc * 512:(sc + 1) * 512], ps)
        if mixed:
            p1pool.release()
            xk_s = x32_s       # scores k-side operand
        else:
            xk_s = x16_s       # fp16 scores read x16 directly (persists)

        # ---- phase 2: attention ------------------------------------------
        with tc.tile_pool(name="p2_strip", bufs=3) as strip_pool, \
             tc.tile_pool(name="p2_exp", bufs=3) as exp_pool, \
             tc.tile_pool(name="p2_expT", bufs=3) as expT_pool, \
             tc.tile_pool(name="p2_stat", bufs=6) as stat_pool, \
             tc.tile_pool(name="p2_acc", bufs=2) as acc_pool, \
             tc.tile_pool(name="p2_ps_s", bufs=4, space="PSUM") as ps_s_pool, \
             tc.tile_pool(name="p2_ps_t", bufs=2, space="PSUM") as ps_t_pool, \
             tc.tile_pool(name="p2_ps_o", bufs=2, space="PSUM") as ps_o_pool:
            acc = None
            for qt in range(ST - 1, -1, -1):
                L = qt + 1
                cols = L * P
                nchn = _ceil_div(cols, 512)
                strip = strip_pool.tile([P, S], f32, tag="strip", name="strip")
                for nch in range(nchn):
                    w = min(512, cols - nch * 512)
                    # f32r needs moving >= 256 for full rate; pad short
                    # tails with throwaway columns
                    wp = max(w, 256) if mixed else w
                    ps = ps_s_pool.tile([P, 512], f32, tag="ps_s", name="ps_s")
                    for dt_ in range(DT):
                        nc.tensor.matmul(
                            ps[:, :wp],
                            t_s[:, dt_, qt * P:(qt + 1) * P],
                            xk_s[:, nch, dt_, 0:wp],
                            start=(dt_ == 0),
                            stop=(dt_ == DT - 1),
                        )
                    # strip = mask - scores (masked slots become +1e10,
                    # transparent to the min-reduce below)
                    in1 = zcm[:, 640 - w:640] if nch == nchn - 1 else zcm[:, 0:w]
                    nc.vector.tensor_sub(
                        strip[:, nch * 512:nch * 512 + w], in1, ps[:, :w])
                nmax = stat_pool.tile([P, 1], f32, tag="nmax", name="nmax")
                nc.vector.tensor_reduce(
                    nmax, strip[:, :cols],
                    axis=mybir.AxisListType.X, op=mybir.AluOpType.min,
                )
                rsum = stat_pool.tile([P, 1], f32, tag="rsum", name="rsum")
                exp16 = exp_pool.tile([P, S], f16, tag="exp16", name="exp16")
                # exp(in*-1 + (-max)) = exp(scores + mask - max)
                nc.scalar.activation(
                    exp16[:, :cols], strip[:, :cols],
                    mybir.ActivationFunctionType.Exp,
                    bias=nmax, scale=-1.0, accum_out=rsum,
                )
                rinv = stat_pool.tile([P, 1], f32, tag="rinv", name="rinv")
                nc.vector.reciprocal(rinv, rsum)
                expT = expT_pool.tile([P, ST, P], f16, tag="expT", name="expT")
                for j0 in range(0, L, 4):
                    jn = min(4, L - j0)
                    pst = ps_t_pool.tile([P, 512], f16, tag="ps_t", name="ps_t")
                    for i in range(jn):
                        nc.tensor.transpose(
                            pst[:, i * P:(i + 1) * P],
                            exp16[:, (j0 + i) * P:(j0 + i + 1) * P],
                            ident16)
                    nc.scalar.copy(expT[:, j0:j0 + jn, :], pst[:, :jn * P])
                if qt % 2 == 1:
                    acc = acc_pool.tile([P, 2, H], f16, tag="acc", name="acc")
                for hc in range(2):
                    pso = ps_o_pool.tile([P, 384], f32, tag="ps_o", name="ps_o")
                    for j in range(L):
                        nc.tensor.matmul(
                            pso,
                            expT[:, j, :],
                            V[:, j, hc * 384:(hc + 1) * 384],
                            start=(j == 0),
                            stop=(j == L - 1),
                        )
                    nc.vector.tensor_scalar_mul(
                        acc[:, qt % 2, hc * 384:(hc + 1) * 384], pso, rinv)
                if qt % 2 == 0:
                    nc.sync.dma_start(
                        out=out_d[:, qt:qt + 2, :], in_=acc)

    nc.finalize()
    return nc


_NC_CACHE = None


def _get_nc():
    global _NC_CACHE
    if _NC_CACHE is None:
        _NC_CACHE = build_nc()
    return _NC_CACHE


def _tile_rows(a):
    """[D, N] -> [128, D//128, N] (partition-major SBUF layout)."""
    d, n = a.shape
    return np.ascontiguousarray(a.reshape(d // P, P, n).transpose(1, 0, 2))


def make_in_maps(x, Wq, Wk, Wv):
    M = (Wq.astype(np.float64) @ Wk.astype(np.float64).T).astype(np.float32)
    m16 = _tile_rows(M.astype(np.float16))
    wv16 = _tile_rows(Wv.astype(np.float16))
    wv16 = np.ascontiguousarray(
        wv16.reshape(P, DT, 2, 384).transpose(2, 0, 1, 3))
    mixed = SCHEME == "mixed"
    in_maps = []
    for c in range(N_CORES):
        xT = np.ascontiguousarray(x[c].T)  # [D, S] f32
        xc32 = np.ascontiguousarray(
            xT.reshape(DT, P, SC, 512).transpose(2, 1, 0, 3))
        im = {"x16": xc32.astype(np.float16), "m16": m16, "wv": wv16}
        if mixed:
            im["x32"] = xc32
        in_maps.append(im)
    return in_maps


def kernel(x, Wq, Wk, Wv):
    x = np.asarray(x, dtype=np.float32)
    Wq = np.asarray(Wq, dtype=np.float32)
    Wk = np.asarray(Wk, dtype=np.float32)
    Wv = np.asarray(Wv, dtype=np.float32)

    nc = _get_nc()
    in_maps = make_in_maps(x, Wq, Wk, Wv)
    res = run_bass_kernel_spmd(nc, in_maps, list(range(N_CORES)))
    out = np.stack(
        [res.results[c]["out"].transpose(1, 0, 2).reshape(S, H)
         for c in range(N_CORES)], axis=0)
    return out.astype(np.float32)
